# revision 1
# baseline (speedup 1.0000x reference)
"""CPMAnt transformer block on 8 TRN2 NeuronCores.

Sharding (Megatron-style): core c owns 4 attention heads (q/k/v/o slices) and
1280 FFN columns (w0/w1/w_out slices). Activations are kept feature-major
(D on partitions) on device. Cross-core comms: 4 chunked AllReduces of the
attention output (feeds the FFN everywhere) and 4 chunked ReduceScatters of
the combined (attention + FFN) partials (feeds each core's 512-row slice of
the final residual). Host folds RMSNorm weights / softmax scale / mask into
the weights and position bias, transposes activations, and concatenates the
8 per-core (512, 2048) outputs.
"""

import math

import numpy as np

S = 2048
D = 4096
H = 32
DH = 128
FF = 10240
NCORES = 8
P = 128
HPC = H // NCORES            # 4 heads per core
WPC = HPC * DH               # 512   per-core qkv width
FPC = FF // NCORES           # 1280  per-core ff width
FCC = FPC // P               # 10
DC = D // P                  # 32
SCN = 4                      # S chunks
SCW = S // SCN               # 512
KC = S // P                  # 16 key chunks
EPS = 1e-6

_CACHE = {}


def _build(stages="all"):
    import concourse.bacc as bacc
    import concourse.tile as tile
    from concourse import mybir

    f32 = mybir.dt.float32
    bf = mybir.dt.bfloat16
    AF = mybir.ActivationFunctionType
    ALU = mybir.AluOpType
    AX = mybir.AxisListType
    RG = [list(range(NCORES))]

    nc = bacc.Bacc(None, num_devices=NCORES)

    hT = nc.dram_tensor("hT", [D, S], f32, kind="ExternalInput")
    h_own = nc.dram_tensor("h_own", [WPC, S], f32, kind="ExternalInput")
    wq = nc.dram_tensor("wq", [D, WPC], bf, kind="ExternalInput")
    wk = nc.dram_tensor("wk", [D, WPC], bf, kind="ExternalInput")
    wv = nc.dram_tensor("wv", [D, WPC], bf, kind="ExternalInput")
    wo = nc.dram_tensor("wo", [4, HPC, P, 8 * P], bf, kind="ExternalInput")
    pb = nc.dram_tensor("pb", [HPC, KC, SCN, P, SCW], bf, kind="ExternalInput")
    w0 = nc.dram_tensor("w0", [FCC, P, DC, P], bf, kind="ExternalInput")
    w1 = nc.dram_tensor("w1", [FCC, P, DC, P], bf, kind="ExternalInput")
    wout = nc.dram_tensor("wout", [8, FCC, P, 4 * P], bf, kind="ExternalInput")
    eye = nc.dram_tensor("eye", [P, P], bf, kind="ExternalInput")
    ones = nc.dram_tensor("ones", [P, P], bf, kind="ExternalInput")
    out = nc.dram_tensor("out", [WPC, S], f32, kind="ExternalOutput")

    from contextlib import ExitStack

    with tile.TileContext(nc) as tc:
        with ExitStack() as ctx:
            ep = ctx.enter_context
            dram = ep(tc.tile_pool(name="dram", bufs=1, space="DRAM"))
            singles = ep(tc.tile_pool(name="singles", bufs=1))
            xarena = ep(tc.tile_pool(name="xarena", bufs=1))
            hpool = ep(tc.tile_pool(name="hstr", bufs=3))
            sqpool = ep(tc.tile_pool(name="sqp", bufs=3))
            rpool = ep(tc.tile_pool(name="rbc", bufs=2))
            wpool = ep(tc.tile_pool(name="wstr", bufs=3))
            cpool = ep(tc.tile_pool(name="cpy", bufs=4))
            apool = ep(tc.tile_pool(name="att", bufs=2))
            ppool = ep(tc.tile_pool(name="pexp", bufs=2))
            spool = ep(tc.tile_pool(name="tiny", bufs=4))
            bpool = ep(tc.tile_pool(name="big16", bufs=2))
            atpool = ep(tc.tile_pool(name="atn", bufs=2))
            w01pool = ep(tc.tile_pool(name="w01", bufs=1))
            woutpool = ep(tc.tile_pool(name="wou", bufs=3))
            outpool = ep(tc.tile_pool(name="outp", bufs=2))
            psum = ep(tc.tile_pool(name="ps", bufs=4, space="PSUM"))
            # ---- DRAM scratch ----
            qT_d = dram.tile([HPC, SCN, P, SCW], bf, tag="qt")
            kT_d = dram.tile([HPC, P, S], bf, tag="kt")
            v_d = dram.tile([HPC, KC, P, DH], bf, tag="vd")
            arin = [dram.tile([D, SCW], bf, tag=f"arin{j}", name=f"arin{j}") for j in range(SCN)]
            arout = [
                dram.tile([D, SCW], bf, tag=f"arout{j}", name=f"arout{j}",
                          addr_space="Shared")
                for j in range(SCN)
            ]
            rsin = [dram.tile([D, SCW], bf, tag=f"rsin{j}", name=f"rsin{j}") for j in range(SCN)]
            rsout = [dram.tile([WPC, SCW], bf, tag=f"rsout{j}", name=f"rsout{j}") for j in range(SCN)]

            eye_sb = singles.tile([P, P], bf)
            nc.sync.dma_start(out=eye_sb[:], in_=eye[:])
            ones_sb = singles.tile([P, P], bf)
            nc.sync.dma_start(out=ones_sb[:], in_=ones[:])
            eps_t = singles.tile([P, 1], f32)
            nc.vector.memset(eps_t[:], EPS)

            # ================= stage 1+2: rmsnorm1 + QKV, per S-chunk =========
            for j in range(SCN):
                xj = xarena.tile([P, DC, SCW], bf, tag="acts")
                ss = psum.tile([P, SCW], f32, tag="psA")
                for d in range(DC):
                    hld = hpool.tile([P, SCW], f32)
                    nc.sync.dma_start(
                        out=hld[:], in_=hT[d * P:(d + 1) * P, j * SCW:(j + 1) * SCW]
                    )
                    nc.vector.tensor_copy(out=xj[:, d, :], in_=hld[:])
                    sq = sqpool.tile([P, SCW], bf)
                    nc.vector.tensor_mul(sq[:], hld[:], hld[:])
                    nc.tensor.matmul(
                        ss[:], lhsT=ones_sb[:], rhs=sq[:],
                        start=(d == 0), stop=(d == DC - 1),
                    )
                rbc = rpool.tile([P, SCW], f32)
                nc.scalar.activation(
                    out=rbc[:], in_=ss[:], func=AF.Sqrt, bias=eps_t[:], scale=1.0 / D
                )
                nc.vector.reciprocal(out=rbc[:], in_=rbc[:])
                for d in range(DC):
                    nc.vector.tensor_mul(xj[:, d, :], xj[:, d, :], rbc[:])

                # ---- Q^T then K^T: 4 psum banks each, accumulate over d ----
                for name, wsrc, dst in (("q", wq, qT_d), ("k", wk, kT_d)):
                    psb = [psum.tile([P, SCW], f32, tag="psA", name=f"ps_{name}{h2}") for h2 in range(HPC)]
                    for d in range(DC):
                        wd = wpool.tile([P, WPC], bf)
                        nc.sync.dma_start(out=wd[:], in_=wsrc[d * P:(d + 1) * P, :])
                        for h in range(HPC):
                            nc.tensor.matmul(
                                psb[h][:], lhsT=wd[:, h * DH:(h + 1) * DH],
                                rhs=xj[:, d, :], start=(d == 0), stop=(d == DC - 1),
                            )
                    for h in range(HPC):
                        cp = cpool.tile([P, SCW], bf)
                        nc.vector.tensor_copy(out=cp[:], in_=psb[h][:])
                        if dst is qT_d:
                            nc.sync.dma_start(out=dst[h, j], in_=cp[:])
                        else:
                            nc.sync.dma_start(
                                out=dst[h, :, j * SCW:(j + 1) * SCW], in_=cp[:]
                            )

                # ---- V (natural layout): 4 psum banks over d ----
                psv = [psum.tile([P, WPC], f32, tag="psA", name=f"psv{sl2}") for sl2 in range(SCW // P)]
                for d in range(DC):
                    wvd = wpool.tile([P, WPC], bf)
                    nc.sync.dma_start(out=wvd[:], in_=wv[d * P:(d + 1) * P, :])
                    for sl in range(SCW // P):
                        nc.tensor.matmul(
                            psv[sl][:], lhsT=xj[:, d, sl * P:(sl + 1) * P],
                            rhs=wvd[:], start=(d == 0), stop=(d == DC - 1),
                        )
                for sl in range(SCW // P):
                    cp = cpool.tile([P, WPC], bf)
                    nc.vector.tensor_copy(out=cp[:], in_=psv[sl][:])
                    for h in range(HPC):
                        nc.sync.dma_start(
                            out=v_d[h, j * (SCW // P) + sl],
                            in_=cp[:, h * DH:(h + 1) * DH],
                        )

            # ================= stage 3+4: attention + wo + AllReduce ==========
            for qg in (range(SCN) if stages in ("all", "qkv+attn", "noffn") else []):
                attnT = atpool.tile([P, HPC, SCW], bf)
                for h in range(HPC):
                    qt_h = apool.tile([P, SCW], bf, tag="qt")
                    nc.sync.dma_start(out=qt_h[:], in_=qT_d[h, qg])
                    kt_h = apool.tile([P, S], bf, tag="kt")
                    nc.sync.dma_start(out=kt_h[:], in_=kT_d[h])
                    v_h = apool.tile([P, KC, DH], bf, tag="vh")
                    nc.sync.dma_start(
                        out=v_h[:], in_=v_d[h].rearrange("kc p f -> p kc f")
                    )
                    pt_sb = bpool.tile([P, KC, SCW], bf, tag="big")
                    for qc in range(SCW // P):
                        pe = ppool.tile([P, S], bf)
                        sums = spool.tile([P, 4], f32, tag="sums")
                        for k4 in range(SCN):
                            pss = psum.tile([P, SCW], f32, tag="psA")
                            nc.tensor.matmul(
                                pss[:], lhsT=qt_h[:, qc * P:(qc + 1) * P],
                                rhs=kt_h[:, k4 * SCW:(k4 + 1) * SCW],
                                start=True, stop=True,
                            )
                            pbt = cpool.tile([P, SCW], bf, tag="pb", bufs=4)
                            nc.sync.dma_start(
                                out=pbt[:], in_=pb[h, qg * (SCW // P) + qc, k4]
                            )
                            sadd = sqpool.tile([P, SCW], f32, tag="sadd")
                            nc.vector.tensor_add(sadd[:], pss[:], pbt[:])
                            nc.scalar.activation(
                                out=pe[:, k4 * SCW:(k4 + 1) * SCW], in_=sadd[:],
                                func=AF.Exp, accum_out=sums[:, k4:k4 + 1],
                            )
                        stot = spool.tile([P, 1], f32, tag="stot")
                        nc.vector.tensor_reduce(
                            stot[:], sums[:], axis=AX.X, op=ALU.add
                        )
                        rs = spool.tile([P, 1], f32, tag="rs")
                        nc.vector.reciprocal(out=rs[:], in_=stot[:])
                        diag = spool.tile([P, P], bf, tag="diag")
                        nc.vector.tensor_scalar_mul(diag[:], eye_sb[:], rs[:])
                        # transpose+normalize: PT[k, q] = P[q, k] / s_q
                        pspt = psum.tile([P, KC, P], f32, tag="pt4", bufs=1)
                        for kc in range(KC):
                            nc.tensor.matmul(
                                pspt[:, kc, :], lhsT=pe[:, kc * P:(kc + 1) * P],
                                rhs=diag[:], start=True, stop=True,
                            )
                        nc.vector.tensor_copy(
                            out=pt_sb[:, :, qc * P:(qc + 1) * P], in_=pspt[:]
                        )
                    psav = psum.tile([P, SCW], f32, tag="psA")
                    for kc in range(KC):
                        nc.tensor.matmul(
                            psav[:], lhsT=v_h[:, kc, :], rhs=pt_sb[:, kc, :],
                            start=(kc == 0), stop=(kc == KC - 1),
                        )
                    nc.vector.tensor_copy(out=attnT[:, h, :], in_=psav[:])

                # ---- wo partials for this S chunk ----
                for dg in (range(4) if stages in ("all", "noffn") else []):
                    wo_sbs = []
                    for h in range(HPC):
                        wo_h = wpool.tile([P, 8 * P], bf, tag="wo", bufs=8,
                                          name=f"wo_h{h}")
                        nc.sync.dma_start(out=wo_h[:], in_=wo[dg, h])
                        wo_sbs.append(wo_h)
                    for di in range(8):
                        dcc = dg * 8 + di
                        pswo = psum.tile([P, SCW], f32, tag="psA")
                        for h in range(HPC):
                            nc.tensor.matmul(
                                pswo[:], lhsT=wo_sbs[h][:, di * P:(di + 1) * P],
                                rhs=attnT[:, h, :],
                                start=(h == 0), stop=(h == HPC - 1),
                            )
                        wcp = cpool.tile([P, SCW], bf)
                        nc.vector.tensor_copy(out=wcp[:], in_=pswo[:])
                        nc.sync.dma_start(
                            out=arin[qg][dcc * P:(dcc + 1) * P, :], in_=wcp[:]
                        )
                if stages in ("all", "noffn"):
                    nc.gpsimd.collective_compute(
                        "AllReduce", ALU.add, replica_groups=RG,
                        ins=[arin[qg][:]], outs=[arout[qg][:]],
                    )

            # ============ stage 5+6: h1, rmsnorm2, FFN, ReduceScatter =========
            for sc in (range(SCN) if stages in ("all", "ffn") else []):
                h1 = xarena.tile([P, DC, SCW], bf, tag="acts")
                ss2 = psum.tile([P, SCW], f32, tag="psA")
                for d in range(DC):
                    hld = hpool.tile([P, SCW], f32)
                    nc.sync.dma_start(
                        out=hld[:], in_=hT[d * P:(d + 1) * P, sc * SCW:(sc + 1) * SCW]
                    )
                    ars = cpool.tile([P, SCW], bf, tag="ars", bufs=2)
                    if stages == "ffn":
                        nc.vector.tensor_copy(out=ars[:], in_=hld[:])
                    else:
                        nc.sync.dma_start(
                            out=ars[:], in_=arout[sc][d * P:(d + 1) * P, :]
                        )
                    nc.vector.tensor_add(h1[:, d, :], hld[:], ars[:])
                    sq = sqpool.tile([P, SCW], bf)
                    nc.vector.tensor_mul(sq[:], h1[:, d, :], h1[:, d, :])
                    nc.tensor.matmul(
                        ss2[:], lhsT=ones_sb[:], rhs=sq[:],
                        start=(d == 0), stop=(d == DC - 1),
                    )
                rbc2 = rpool.tile([P, SCW], f32)
                nc.scalar.activation(
                    out=rbc2[:], in_=ss2[:], func=AF.Sqrt, bias=eps_t[:], scale=1.0 / D
                )
                nc.vector.reciprocal(out=rbc2[:], in_=rbc2[:])
                for d in range(DC):
                    nc.vector.tensor_mul(h1[:, d, :], h1[:, d, :], rbc2[:])

                # ---- gated FFN ----
                ffT = bpool.tile([P, FCC, SCW], bf, tag="big")
                for fc in range(FCC):
                    w0b = w01pool.tile([P, DC, P], bf, tag="w0")
                    nc.sync.dma_start(out=w0b[:], in_=w0[fc])
                    w1b = w01pool.tile([P, DC, P], bf, tag="w1")
                    nc.sync.dma_start(out=w1b[:], in_=w1[fc])
                    psg = psum.tile([P, SCW], f32, tag="psA")
                    psu = psum.tile([P, SCW], f32, tag="psA")
                    for d in range(DC):
                        nc.tensor.matmul(
                            psg[:], lhsT=w0b[:, d, :], rhs=h1[:, d, :],
                            start=(d == 0), stop=(d == DC - 1),
                        )
                        nc.tensor.matmul(
                            psu[:], lhsT=w1b[:, d, :], rhs=h1[:, d, :],
                            start=(d == 0), stop=(d == DC - 1),
                        )
                    gel = sqpool.tile([P, SCW], bf, tag="gel")
                    nc.scalar.activation(out=gel[:], in_=psg[:], func=AF.Gelu)
                    nc.vector.tensor_mul(ffT[:, fc, :], psu[:], gel[:])

                # ---- w_out partials + fold in attention partial ----
                for dg in range(8):
                    ps2 = [psum.tile([P, SCW], f32, tag="psA", name=f"ps2_{di2}") for di2 in range(4)]
                    for fc in range(FCC):
                        wob = woutpool.tile([P, 4 * P], bf)
                        nc.sync.dma_start(out=wob[:], in_=wout[dg, fc])
                        for di in range(4):
                            nc.tensor.matmul(
                                ps2[di][:], lhsT=wob[:, di * P:(di + 1) * P],
                                rhs=ffT[:, fc, :],
                                start=(fc == 0), stop=(fc == FCC - 1),
                            )
                    for di in range(4):
                        dcc = dg * 4 + di
                        rcp = cpool.tile([P, SCW], bf, tag="rcp", bufs=2)
                        if stages == "ffn":
                            nc.vector.tensor_copy(out=rcp[:], in_=ps2[di][:])
                        else:
                            arp = cpool.tile([P, SCW], bf, tag="arp", bufs=2)
                            nc.sync.dma_start(
                                out=arp[:], in_=arin[sc][dcc * P:(dcc + 1) * P, :]
                            )
                            nc.vector.tensor_add(rcp[:], ps2[di][:], arp[:])
                        nc.sync.dma_start(
                            out=rsin[sc][dcc * P:(dcc + 1) * P, :], in_=rcp[:]
                        )
                nc.gpsimd.collective_compute(
                    "ReduceScatter", ALU.add, replica_groups=RG,
                    ins=[rsin[sc][:]], outs=[rsout[sc][:]],
                )

            # ================= stage 7: final residual, output ================
            for sc in (range(SCN) if stages == "all" else [0]):
                for ol in range(WPC // P):
                    hot = hpool.tile([P, SCW], f32)
                    nc.sync.dma_start(
                        out=hot[:],
                        in_=h_own[ol * P:(ol + 1) * P, sc * SCW:(sc + 1) * SCW],
                    )
                    ot = outpool.tile([P, SCW], f32)
                    if stages in ("all", "ffn"):
                        rst = cpool.tile([P, SCW], bf, tag="rst", bufs=2)
                        nc.sync.dma_start(
                            out=rst[:], in_=rsout[sc][ol * P:(ol + 1) * P, :]
                        )
                        nc.vector.tensor_add(ot[:], hot[:], rst[:])
                    else:
                        nc.vector.tensor_copy(out=ot[:], in_=hot[:])
                    nc.sync.dma_start(
                        out=out[ol * P:(ol + 1) * P, sc * SCW:(sc + 1) * SCW],
                        in_=ot[:],
                    )

    nc.finalize()
    return nc


def _prep_in_maps(inputs):
    import ml_dtypes

    bf16 = ml_dtypes.bfloat16
    hid = np.ascontiguousarray(np.asarray(inputs["hidden_states"], np.float32)[0])
    mask = np.asarray(inputs["attention_mask"])[0]
    pbias = np.asarray(inputs["position_bias"], np.float32)[0]
    ln_a = np.asarray(inputs["ln_attn_w"], np.float32)
    ln_f = np.asarray(inputs["ln_ffn_w"], np.float32)
    wq = np.asarray(inputs["wq"], np.float32)
    wk = np.asarray(inputs["wk"], np.float32)
    wv = np.asarray(inputs["wv"], np.float32)
    wo = np.asarray(inputs["wo"], np.float32)
    w0 = np.asarray(inputs["w0"], np.float32)
    w1 = np.asarray(inputs["w1"], np.float32)
    w_out = np.asarray(inputs["w_out"], np.float32)

    hT = np.ascontiguousarray(hid.T)                          # (D, S) f32
    wq_f = (ln_a[:, None] * wq * (DH ** -0.5)).astype(bf16)
    wk_f = (ln_a[:, None] * wk).astype(bf16)
    wv_f = (ln_a[:, None] * wv).astype(bf16)
    wo_f = wo.astype(bf16)
    w0_f = (ln_f[:, None] * w0).astype(bf16)
    w1_f = (ln_f[:, None] * w1).astype(bf16)
    wout_f = w_out.astype(bf16)
    if mask.all():
        pb_m = pbias.astype(bf16)
    else:
        pb_m = np.where(mask[None], pbias, np.float32(-1e30)).astype(bf16)

    eye = np.eye(P, dtype=bf16)
    ones = np.ones((P, P), dtype=bf16)

    in_maps = []
    for c in range(NCORES):
        ws = slice(c * WPC, (c + 1) * WPC)
        fs = slice(c * FPC, (c + 1) * FPC)
        # wo: (WPC, D) -> (4 dgrp, HPC, P, 8*P): [dg,h,p,f] = wo[h*128+p, dg*1024+f]
        wo_c = wo_f[ws, :].reshape(HPC, P, 4, 8 * P).transpose(2, 0, 1, 3)
        # pb: (HPC, S, S) -> (HPC, KC qc, SCN k4, P, SCW)
        pb_c = pb_m[c * HPC:(c + 1) * HPC].reshape(HPC, KC, P, SCN, SCW)
        pb_c = pb_c.transpose(0, 1, 3, 2, 4)
        # w0/w1: (D, FPC) -> (FCC, P, DC, P): [fc,p,d,f] = w[d*128+p, fc*128+f]
        w0_c = w0_f[:, fs].reshape(DC, P, FCC, P).transpose(2, 1, 0, 3)
        w1_c = w1_f[:, fs].reshape(DC, P, FCC, P).transpose(2, 1, 0, 3)
        # wout: (FPC, D) -> (8 dg, FCC, P, 4*P): [dg,fc,p,f] = wout[fc*128+p, dg*512+f]
        wout_c = wout_f[fs, :].reshape(FCC, P, 8, 4 * P).transpose(2, 0, 1, 3)
        in_maps.append({
            "hT": hT,
            "h_own": np.ascontiguousarray(hT[ws]),
            "wq": np.ascontiguousarray(wq_f[:, ws]),
            "wk": np.ascontiguousarray(wk_f[:, ws]),
            "wv": np.ascontiguousarray(wv_f[:, ws]),
            "wo": np.ascontiguousarray(wo_c),
            "pb": np.ascontiguousarray(pb_c),
            "w0": np.ascontiguousarray(w0_c),
            "w1": np.ascontiguousarray(w1_c),
            "wout": np.ascontiguousarray(wout_c),
            "eye": eye,
            "ones": ones,
        })
    return in_maps


def get_nc(stages="all"):
    if stages not in _CACHE:
        _CACHE[stages] = _build(stages)
    return _CACHE[stages]


def kernel(**inputs):
    from concourse.bass_utils import run_bass_kernel_spmd

    nc = get_nc()
    in_maps = _prep_in_maps(inputs)
    res = run_bass_kernel_spmd(nc, in_maps, core_ids=list(range(NCORES)))
    parts = [res.results[c]["out"] for c in range(NCORES)]   # each (WPC, S)
    full_T = np.concatenate(parts, axis=0)                    # (D, S)
    out = np.ascontiguousarray(full_T.T)[None]                # (1, S, D)
    return out.astype(np.float32)



# revision 2
# speedup vs baseline: 1.1995x; 1.1995x over previous
"""CPMAnt transformer block on 8 TRN2 NeuronCores (Megatron-style TP).

Core c owns 4 attention heads and 1280 FFN columns. Activations are
feature-major (D on partitions). QKV / attention-out / AV / softmax-sum /
sum-of-squares matmuls run in fp8 (e4m3 / e5m2) DoubleRow mode (2 k-tiles
per instruction = 2x PE throughput); scores and the FFN run in bf16.
q/k/v and attention probabilities never leave SBUF. Scores are computed
k-major (out[k, q]) so no PE transposes are needed; the softmax
denominator comes from an fp8 ones-matmul and normalization is folded
into the attn output copy. Cross-core comms: 4 chunked AllReduces of the
attention output and 4 chunked ReduceScatters of (attention + FFN)
partials, as in the reference Megatron schedule.
"""

import math

import numpy as np

S = 2048
D = 4096
H = 32
DH = 128
FF = 10240
NCORES = 8
P = 128
HPC = H // NCORES            # 4 heads per core
WPC = HPC * DH               # 512   per-core qkv width
FPC = FF // NCORES           # 1280  per-core ff width
FCC = FPC // P               # 10
DC = D // P                  # 32
DCH = DC // 2                # 16  (d-tiles per half chunk)
SCN = 4                      # S chunks
SCW = S // SCN               # 512
KC = S // P                  # 16 key chunks
EPS = 1e-6

# fp8 weight scales (powers of two; descaled at psum copy-out)
S_WQ = 256.0                 # wq folded with 1/sqrt(DH): std ~0.0014
S_WK = 16.0
S_WV = 16.0
S_WO = 16.0
S_QS = 4.0                   # q stored as 4*q (e4m3); pb pre-scaled by 4 on host
S_VS = 8.0                   # v stored as 8*v; cancels with attn fp8 scale

_CACHE = {}


def _build():
    import concourse.bacc as bacc
    import concourse.tile as tile
    from concourse import mybir

    f32 = mybir.dt.float32
    bf = mybir.dt.bfloat16
    e4 = mybir.dt.float8e4
    e5 = mybir.dt.float8e5
    AF = mybir.ActivationFunctionType
    ALU = mybir.AluOpType
    DR = mybir.MatmulPerfMode.DoubleRow
    RG = [list(range(NCORES))]

    nc = bacc.Bacc(None, num_devices=NCORES)

    hT = nc.dram_tensor("hT", [DC, P, S], bf, kind="ExternalInput")
    h_own = nc.dram_tensor("h_own", [4, P, S], f32, kind="ExternalInput")
    wq = nc.dram_tensor("wq", [P, DCH, 2, WPC], e4, kind="ExternalInput")
    wk = nc.dram_tensor("wk", [P, DCH, 2, WPC], e4, kind="ExternalInput")
    wv = nc.dram_tensor("wv", [P, DCH, 2, WPC], e4, kind="ExternalInput")
    wo = nc.dram_tensor("wo", [P, 2, 2, D], e4, kind="ExternalInput")
    pbT = nc.dram_tensor("pbT", [HPC, SCN, 2, P, 8, SCW], bf, kind="ExternalInput")
    w0 = nc.dram_tensor("w0", [FCC, P, DC, P], bf, kind="ExternalInput")
    w1 = nc.dram_tensor("w1", [FCC, P, DC, P], bf, kind="ExternalInput")
    wout = nc.dram_tensor("wout", [8, P, FCC, 4 * P], bf, kind="ExternalInput")
    ones4 = nc.dram_tensor("ones4", [P, 2, P], e4, kind="ExternalInput")
    ones5 = nc.dram_tensor("ones5", [P, 2, P], e5, kind="ExternalInput")
    out = nc.dram_tensor("out", [WPC, S], f32, kind="ExternalOutput")

    from contextlib import ExitStack

    with tile.TileContext(nc) as tc:
        with ExitStack() as ctx:
            ep = ctx.enter_context
            dram = ep(tc.tile_pool(name="dram", bufs=1, space="DRAM"))
            singles = ep(tc.tile_pool(name="singles", bufs=1))
            arena = ep(tc.tile_pool(name="arena", bufs=1))
            hstr = ep(tc.tile_pool(name="hstr", bufs=3))
            xarena = ep(tc.tile_pool(name="xarena", bufs=2))
            wstr = ep(tc.tile_pool(name="wstr", bufs=2))
            pbp = ep(tc.tile_pool(name="pbp", bufs=3))
            p5p = ep(tc.tile_pool(name="p5p", bufs=2))
            atp = ep(tc.tile_pool(name="atp", bufs=2))
            sap = ep(tc.tile_pool(name="sap", bufs=4))
            rbp = ep(tc.tile_pool(name="rbp", bufs=2))
            psum = ep(tc.tile_pool(name="ps", bufs=6, space="PSUM"))
            psB = ep(tc.tile_pool(name="psB", bufs=2, space="PSUM"))

            # ---- DRAM scratch for collectives ----
            arin = [dram.tile([DC, P, SCW], bf, tag=f"arin{j}", name=f"arin{j}")
                    for j in range(SCN)]
            arout = [dram.tile([DC, P, SCW], bf, tag=f"arout{j}", name=f"arout{j}",
                               addr_space="Shared") for j in range(SCN)]
            rsin = [dram.tile([DC, P, SCW], bf, tag=f"rsin{j}", name=f"rsin{j}")
                    for j in range(SCN)]
            rsout = [dram.tile([4, P, SCW], bf, tag=f"rsout{j}", name=f"rsout{j}")
                     for j in range(SCN)]

            ones4_sb = singles.tile([P, 2, P], e4)
            nc.sync.dma_start(out=ones4_sb[:], in_=ones4[:])
            ones5_sb = singles.tile([P, 2, P], e5)
            nc.sync.dma_start(out=ones5_sb[:], in_=ones5[:])
            eps_t = singles.tile([P, 1], f32)
            nc.vector.memset(eps_t[:], EPS)

            # persistent SBUF arenas for q/k/v (fp8)
            qT = arena.tile([P, HPC, S], e4, tag="qT")       # [dh, h, s] = 4*q
            kT = arena.tile([P, HPC, S], e4, tag="kT")       # [dh, h, s] = k
            v8 = arena.tile([P, HPC, 8, 2, DH], e4, tag="v8")  # [kp, h, jj, i, dh] = 8*v

            # ============ phase 1: rmsnorm1 + QKV (per S-chunk) ============
            for j in range(SCN):
                cols = slice(j * SCW, (j + 1) * SCW)
                halves = []
                sq8s = []
                ss = psB.tile([P, SCW], f32, tag="pB", name="ss_ps")
                for hf in range(2):
                    hld = hstr.tile([P, DCH, SCW], bf, tag="hstream",
                                    name=f"hld{hf}")
                    nc.sync.dma_start(
                        out=hld[:],
                        in_=hT[hf * DCH:(hf + 1) * DCH, :, cols].rearrange(
                            "d p s -> p d s"),
                    )
                    halves.append(hld)
                    sq8 = p5p.tile([P, DCH, SCW], e4, tag="p5", name=f"sq8{hf}")
                    nc.vector.tensor_mul(sq8[:], hld[:], hld[:])
                    sq8s.append(sq8)
                    for jj in range(DCH // 2):
                        nc.tensor.matmul(
                            ss[:], lhsT=ones4_sb[:],
                            rhs=sq8[:, 2 * jj:2 * jj + 2, :],
                            start=(hf == 0 and jj == 0),
                            stop=(hf == 1 and jj == DCH // 2 - 1),
                            perf_mode=DR,
                        )
                rbc = rbp.tile([P, SCW], f32, tag="rbc")
                nc.scalar.activation(
                    out=rbc[:], in_=ss[:], func=AF.Sqrt, bias=eps_t[:],
                    scale=1.0 / D,
                )
                nc.vector.reciprocal(out=rbc[:], in_=rbc[:])
                x8 = xarena.tile([P, DC, SCW], e4, tag="x8")
                for d in range(DC):
                    nc.vector.tensor_mul(
                        x8[:, d, :], halves[d // DCH][:, d % DCH, :], rbc[:])

                # ---- Q, K (feature-major out [dh, s]) ----
                for name, wsrc, dst, cscale in (
                    ("q", wq, qT, S_QS / S_WQ), ("k", wk, kT, 1.0 / S_WK),
                ):
                    wsb = wstr.tile([P, DCH, 2, WPC], e4, tag="wstream",
                                    name=f"w{name}sb")
                    nc.sync.dma_start(out=wsb[:], in_=wsrc[:])
                    for h in range(HPC):
                        ps = psum.tile([P, SCW], f32, tag="pA",
                                       name=f"ps_{name}{h}")
                        for dp in range(DCH):
                            nc.tensor.matmul(
                                ps[:], lhsT=wsb[:, dp, :, h * DH:(h + 1) * DH],
                                rhs=x8[:, 2 * dp:2 * dp + 2, :],
                                start=(dp == 0), stop=(dp == DCH - 1),
                                perf_mode=DR,
                            )
                        nc.scalar.mul(dst[:, h, cols], ps[:], cscale)

                # ---- V (natural layout out [s, dh]) ----
                wvsb = wstr.tile([P, DCH, 2, WPC], e4, tag="wstream", name="wvsb")
                nc.sync.dma_start(out=wvsb[:], in_=wv[:])
                for sl in range(SCW // P):
                    ps = psum.tile([P, WPC], f32, tag="pA", name=f"ps_v{sl}")
                    for dp in range(DCH):
                        nc.tensor.matmul(
                            ps[:], lhsT=x8[:, 2 * dp:2 * dp + 2, sl * P:(sl + 1) * P],
                            rhs=wvsb[:, dp, :, :],
                            start=(dp == 0), stop=(dp == DCH - 1),
                            perf_mode=DR,
                        )
                    kcix = j * (SCW // P) + sl
                    nc.scalar.mul(
                        v8[:, :, kcix // 2, kcix % 2, :],
                        ps[:].rearrange("p (h f) -> p h f", h=HPC),
                        S_VS / S_WV,
                    )

            # ============ phase 2: attention (k-major) + WO + AllReduce ====
            for qg in range(SCN):
                qcols = slice(qg * SCW, (qg + 1) * SCW)
                attnT = atp.tile([P, HPC, SCW], e4, tag="attnT")
                for h in range(HPC):
                    p5 = p5p.tile([P, KC, SCW], e5, tag="p5", name="p5")
                    sums = psB.tile([P, SCW], f32, tag="pB", name="sums_ps")
                    for hf in range(2):
                        pbt = pbp.tile([P, 8, SCW], bf, tag="pbt", name="pbt")
                        nc.sync.dma_start(out=pbt[:], in_=pbT[h, qg, hf])
                        for kk in range(8):
                            kc = hf * 8 + kk
                            pss = psum.tile([P, SCW], f32, tag="pA", name="pss")
                            nc.tensor.matmul(
                                pss[:], lhsT=kT[:, h, kc * P:(kc + 1) * P],
                                rhs=qT[:, h, qcols], start=True, stop=True,
                            )
                            sadd = sap.tile([P, SCW], f32, tag="sadd")
                            nc.vector.tensor_add(sadd[:], pss[:], pbt[:, kk, :])
                            nc.scalar.activation(
                                out=p5[:, kc, :], in_=sadd[:], func=AF.Exp,
                                scale=1.0 / S_QS,
                            )
                    for jj in range(KC // 2):
                        nc.tensor.matmul(
                            sums[:], lhsT=ones5_sb[:],
                            rhs=p5[:, 2 * jj:2 * jj + 2, :],
                            start=(jj == 0), stop=(jj == KC // 2 - 1),
                            perf_mode=DR,
                        )
                    psav = psB.tile([P, SCW], f32, tag="pB", name="psav")
                    for jj in range(KC // 2):
                        nc.tensor.matmul(
                            psav[:], lhsT=v8[:, h, jj, :, :],
                            rhs=p5[:, 2 * jj:2 * jj + 2, :],
                            start=(jj == 0), stop=(jj == KC // 2 - 1),
                            perf_mode=DR,
                        )
                    rs = rbp.tile([P, SCW], f32, tag="rbc", name="rs")
                    nc.vector.reciprocal(out=rs[:], in_=sums[:])
                    nc.vector.tensor_mul(attnT[:, h, :], psav[:], rs[:])

                # ---- WO partials for this S chunk -> arin, AllReduce ----
                wosb = wstr.tile([P, 2, 2, D], e4, tag="wstream", name="wosb")
                nc.sync.dma_start(out=wosb[:], in_=wo[:])
                for dg in range(8):
                    stg = p5p.tile([P, 4, SCW], bf, tag="p5", name="wostg")
                    for di in range(4):
                        dcc = dg * 4 + di
                        ps = psum.tile([P, SCW], f32, tag="pA", name="ps_wo")
                        for hp in range(2):
                            nc.tensor.matmul(
                                ps[:],
                                lhsT=wosb[:, hp, :, dcc * P:(dcc + 1) * P],
                                rhs=attnT[:, 2 * hp:2 * hp + 2, :],
                                start=(hp == 0), stop=(hp == 1),
                                perf_mode=DR,
                            )
                        nc.scalar.mul(stg[:, di, :], ps[:], 1.0 / (S_VS * S_WO))
                    nc.sync.dma_start(
                        out=arin[qg][dg * 4:(dg + 1) * 4, :, :].rearrange(
                            "d p s -> p d s"),
                        in_=stg[:],
                    )
                nc.gpsimd.collective_compute(
                    "AllReduce", ALU.add, replica_groups=RG,
                    ins=[arin[qg][:]], outs=[arout[qg][:]],
                )

            # ============ phase 3: h1, rmsnorm2, FFN, ReduceScatter ========
            for sc in range(SCN):
                cols = slice(sc * SCW, (sc + 1) * SCW)
                halves = []
                ss2 = psB.tile([P, SCW], f32, tag="pB", name="ss2_ps")
                for hf in range(2):
                    h1h = hstr.tile([P, DCH, SCW], bf, tag="hstream",
                                    name=f"h1h{hf}")
                    nc.sync.dma_start(
                        out=h1h[:],
                        in_=hT[hf * DCH:(hf + 1) * DCH, :, cols].rearrange(
                            "d p s -> p d s"),
                    )
                    for qr in range(2):
                        ars = pbp.tile([P, 8, SCW], bf, tag="pbt", name="ars")
                        d0 = hf * DCH + qr * 8
                        nc.sync.dma_start(
                            out=ars[:],
                            in_=arout[sc][d0:d0 + 8, :, :].rearrange(
                                "d p s -> p d s"),
                        )
                        nc.vector.tensor_add(
                            h1h[:, qr * 8:(qr + 1) * 8, :],
                            h1h[:, qr * 8:(qr + 1) * 8, :], ars[:])
                    halves.append(h1h)
                    sq8 = p5p.tile([P, DCH, SCW], e4, tag="p5", name=f"fsq8{hf}")
                    nc.vector.tensor_mul(sq8[:], h1h[:], h1h[:])
                    for jj in range(DCH // 2):
                        nc.tensor.matmul(
                            ss2[:], lhsT=ones4_sb[:],
                            rhs=sq8[:, 2 * jj:2 * jj + 2, :],
                            start=(hf == 0 and jj == 0),
                            stop=(hf == 1 and jj == DCH // 2 - 1),
                            perf_mode=DR,
                        )
                rbc2 = rbp.tile([P, SCW], f32, tag="rbc", name="rbc2")
                nc.scalar.activation(
                    out=rbc2[:], in_=ss2[:], func=AF.Sqrt, bias=eps_t[:],
                    scale=1.0 / D,
                )
                nc.vector.reciprocal(out=rbc2[:], in_=rbc2[:])
                for d in range(DC):
                    y = halves[d // DCH][:, d % DCH, :]
                    nc.vector.tensor_mul(y, y, rbc2[:])

                # ---- gated FFN (bf16) ----
                ffT = xarena.tile([P, FCC, SCW], bf, tag="x8", name="ffT")
                for fc in range(FCC):
                    w0b = wstr.tile([P, DC, P], bf, tag="wstream", name="w0b")
                    nc.sync.dma_start(out=w0b[:], in_=w0[fc])
                    w1b = wstr.tile([P, DC, P], bf, tag="wstream", name="w1b")
                    nc.sync.dma_start(out=w1b[:], in_=w1[fc])
                    psg = psum.tile([P, SCW], f32, tag="pA", name="psg")
                    psu = psum.tile([P, SCW], f32, tag="pA", name="psu")
                    for d in range(DC):
                        y = halves[d // DCH][:, d % DCH, :]
                        nc.tensor.matmul(
                            psg[:], lhsT=w0b[:, d, :], rhs=y,
                            start=(d == 0), stop=(d == DC - 1),
                        )
                        nc.tensor.matmul(
                            psu[:], lhsT=w1b[:, d, :], rhs=y,
                            start=(d == 0), stop=(d == DC - 1),
                        )
                    gel = sap.tile([P, SCW], bf, tag="gel", bufs=2)
                    nc.scalar.activation(out=gel[:], in_=psg[:], func=AF.Gelu)
                    nc.vector.tensor_mul(ffT[:, fc, :], psu[:], gel[:])

                # ---- w_out partials + attention partial -> rsin, RS ----
                for dgp in range(4):
                    arp = pbp.tile([P, 8, SCW], bf, tag="pbt", name="arp")
                    nc.sync.dma_start(
                        out=arp[:],
                        in_=arin[sc][dgp * 8:(dgp + 1) * 8, :, :].rearrange(
                            "d p s -> p d s"),
                    )
                    for dh2 in range(2):
                        dg = dgp * 2 + dh2
                        wob = wstr.tile([P, FCC, 4 * P], bf, tag="wstream",
                                        name="wob")
                        nc.sync.dma_start(out=wob[:], in_=wout[dg])
                        stg = p5p.tile([P, 4, SCW], bf, tag="p5", name="ffstg")
                        for di in range(4):
                            ps = psum.tile([P, SCW], f32, tag="pA", name="ps_o")
                            for fc in range(FCC):
                                nc.tensor.matmul(
                                    ps[:], lhsT=wob[:, fc, di * P:(di + 1) * P],
                                    rhs=ffT[:, fc, :],
                                    start=(fc == 0), stop=(fc == FCC - 1),
                                )
                            nc.vector.tensor_add(
                                stg[:, di, :], ps[:], arp[:, dh2 * 4 + di, :])
                        nc.sync.dma_start(
                            out=rsin[sc][dg * 4:(dg + 1) * 4, :, :].rearrange(
                                "d p s -> p d s"),
                            in_=stg[:],
                        )
                nc.gpsimd.collective_compute(
                    "ReduceScatter", ALU.add, replica_groups=RG,
                    ins=[rsin[sc][:]], outs=[rsout[sc][:]],
                )

            # ============ phase 4: final residual ==========================
            for sc in range(SCN):
                cols = slice(sc * SCW, (sc + 1) * SCW)
                rst = pbp.tile([P, 4, SCW], bf, tag="pbt", name="rst")
                nc.sync.dma_start(
                    out=rst[:], in_=rsout[sc][:].rearrange("o p s -> p o s"))
                hot = hstr.tile([P, 4, SCW], f32, tag="hstream", name="hot")
                nc.sync.dma_start(
                    out=hot[:], in_=h_own[:, :, cols].rearrange("o p s -> p o s"))
                ot = p5p.tile([P, 4, SCW], f32, tag="p5", name="ot")
                nc.vector.tensor_add(ot[:], hot[:], rst[:])
                nc.sync.dma_start(
                    out=out[:, cols].rearrange("(o p) s -> p o s", p=P),
                    in_=ot[:],
                )

    nc.finalize()
    return nc


def _prep_in_maps(inputs):
    import ml_dtypes

    bf16 = ml_dtypes.bfloat16
    e4 = ml_dtypes.float8_e4m3
    e5 = ml_dtypes.float8_e5m2

    def q8(x, s):
        return np.clip(x * s, -240, 240).astype(e4)

    hid = np.ascontiguousarray(np.asarray(inputs["hidden_states"], np.float32)[0])
    mask = np.asarray(inputs["attention_mask"])[0]
    pbias = np.asarray(inputs["position_bias"], np.float32)[0]
    ln_a = np.asarray(inputs["ln_attn_w"], np.float32)
    ln_f = np.asarray(inputs["ln_ffn_w"], np.float32)
    wq = np.asarray(inputs["wq"], np.float32)
    wk = np.asarray(inputs["wk"], np.float32)
    wv = np.asarray(inputs["wv"], np.float32)
    wo = np.asarray(inputs["wo"], np.float32)
    w0 = np.asarray(inputs["w0"], np.float32)
    w1 = np.asarray(inputs["w1"], np.float32)
    w_out = np.asarray(inputs["w_out"], np.float32)

    hT = np.ascontiguousarray(hid.T)                          # (D, S) f32
    hT_bf = hT.reshape(DC, P, S).astype(bf16)
    wq_f = q8(ln_a[:, None] * wq * (DH ** -0.5), S_WQ)
    wk_f = q8(ln_a[:, None] * wk, S_WK)
    wv_f = q8(ln_a[:, None] * wv, S_WV)
    wo_f = q8(wo, S_WO)
    w0_f = (ln_f[:, None] * w0).astype(bf16)
    w1_f = (ln_f[:, None] * w1).astype(bf16)
    wout_f = w_out.astype(bf16)
    if mask.all():
        pb_m = pbias * S_QS
    else:
        pb_m = np.where(mask[None], pbias * S_QS, np.float32(-1e30))
    # transposed position bias: [H, S_k, S_q] -> per-core [HPC,SCN,2,P,8,SCW]
    pbT_full = np.ascontiguousarray(pb_m.transpose(0, 2, 1)).astype(bf16)

    ones4 = np.ones((P, 2, P), dtype=e4)
    ones5 = np.ones((P, 2, P), dtype=e5)

    def wqkv_layout(w):                # (D, WPC) -> (P, DCH, 2, WPC)
        return np.ascontiguousarray(
            w.reshape(DCH, 2, P, WPC).transpose(2, 0, 1, 3))

    in_maps = []
    for c in range(NCORES):
        ws = slice(c * WPC, (c + 1) * WPC)
        fs = slice(c * FPC, (c + 1) * FPC)
        # wo: rows (2hp+i)*128+p of this core's (WPC, D) slice
        wo_c = np.ascontiguousarray(
            wo_f[ws, :].reshape(2, 2, P, D).transpose(2, 0, 1, 3))
        pb_c = pbT_full[c * HPC:(c + 1) * HPC]                # (HPC, S_k, S_q)
        pb_c = pb_c.reshape(HPC, 2, 8, P, SCN, SCW).transpose(0, 4, 1, 3, 2, 5)
        w0_c = w0_f[:, fs].reshape(DC, P, FCC, P).transpose(2, 1, 0, 3)
        w1_c = w1_f[:, fs].reshape(DC, P, FCC, P).transpose(2, 1, 0, 3)
        wout_c = wout_f[fs, :].reshape(FCC, P, 8, 4 * P).transpose(2, 1, 0, 3)
        in_maps.append({
            "hT": hT_bf,
            "h_own": np.ascontiguousarray(hT[ws].reshape(4, P, S)),
            "wq": wqkv_layout(wq_f[:, ws]),
            "wk": wqkv_layout(wk_f[:, ws]),
            "wv": wqkv_layout(wv_f[:, ws]),
            "wo": wo_c,
            "pbT": np.ascontiguousarray(pb_c),
            "w0": np.ascontiguousarray(w0_c),
            "w1": np.ascontiguousarray(w1_c),
            "wout": np.ascontiguousarray(wout_c),
            "ones4": ones4,
            "ones5": ones5,
        })
    return in_maps


def get_nc():
    if "nc" not in _CACHE:
        _CACHE["nc"] = _build()
    return _CACHE["nc"]


def kernel(**inputs):
    from concourse.bass_utils import run_bass_kernel_spmd

    nc = get_nc()
    in_maps = _prep_in_maps(inputs)
    res = run_bass_kernel_spmd(nc, in_maps, core_ids=list(range(NCORES)))
    parts = [res.results[c]["out"] for c in range(NCORES)]   # each (WPC, S)
    full_T = np.concatenate(parts, axis=0)                    # (D, S)
    out = np.ascontiguousarray(full_T.T)[None]                # (1, S, D)
    return out.astype(np.float32)


# revision 3
# speedup vs baseline: 1.3708x; 1.1428x over previous
"""CPMAnt transformer block on 8 TRN2 NeuronCores (Megatron-style TP).

Core c owns 4 attention heads and 1280 FFN columns. Activations are
feature-major (D on partitions). QKV / attention-out / AV / softmax-sum /
sum-of-squares matmuls run in fp8 (e4m3 / e5m2) DoubleRow mode (2 k-tiles
per instruction = 2x PE throughput); scores and the FFN run in bf16.
q/k/v and attention probabilities never leave SBUF. Scores are computed
k-major (out[k, q]) so no PE transposes are needed; the softmax
denominator comes from an fp8 ones-matmul and normalization is folded
into the attn output copy. Cross-core comms: 4 chunked AllReduces of the
attention output and 4 chunked ReduceScatters of (attention + FFN)
partials, as in the reference Megatron schedule.
"""

import math

import numpy as np

S = 2048
D = 4096
H = 32
DH = 128
FF = 10240
NCORES = 8
P = 128
HPC = H // NCORES            # 4 heads per core
WPC = HPC * DH               # 512   per-core qkv width
FPC = FF // NCORES           # 1280  per-core ff width
FCC = FPC // P               # 10
DC = D // P                  # 32
DCH = DC // 2                # 16  (d-tiles per half chunk)
SCN = 4                      # S chunks
SCW = S // SCN               # 512
KC = S // P                  # 16 key chunks
EPS = 1e-6

# fp8 weight scales (powers of two; descaled at psum copy-out)
S_WQ = 256.0                 # wq folded with 1/sqrt(DH): std ~0.0014
S_WK = 16.0
S_WV = 16.0
S_WO = 16.0
S_QS = 4.0                   # q stored as 4*q (e4m3); pb pre-scaled by 4 on host
S_VS = 8.0                   # v stored as 8*v; cancels with attn fp8 scale

_CACHE = {}


def _build():
    import concourse.bacc as bacc
    import concourse.tile as tile
    from concourse import mybir

    f32 = mybir.dt.float32
    bf = mybir.dt.bfloat16
    e4 = mybir.dt.float8e4
    e5 = mybir.dt.float8e5
    AF = mybir.ActivationFunctionType
    ALU = mybir.AluOpType
    DR = mybir.MatmulPerfMode.DoubleRow
    RG = [list(range(NCORES))]

    nc = bacc.Bacc(None, num_devices=NCORES)

    hT = nc.dram_tensor("hT", [DC, P, S], bf, kind="ExternalInput")
    h_own = nc.dram_tensor("h_own", [4, P, S], f32, kind="ExternalInput")
    wq = nc.dram_tensor("wq", [P, DCH, 2, WPC], e4, kind="ExternalInput")
    wk = nc.dram_tensor("wk", [P, DCH, 2, WPC], e4, kind="ExternalInput")
    wv = nc.dram_tensor("wv", [P, DCH, 2, WPC], e4, kind="ExternalInput")
    wo = nc.dram_tensor("wo", [P, 2, 2, D], e4, kind="ExternalInput")
    pbT = nc.dram_tensor("pbT", [HPC, SCN, 2, P, 8, SCW], bf, kind="ExternalInput")
    w01 = nc.dram_tensor("w01", [FCC, P, 2, DC, P], bf, kind="ExternalInput")
    wout = nc.dram_tensor("wout", [8, P, FCC, 4 * P], bf, kind="ExternalInput")
    ones4 = nc.dram_tensor("ones4", [P, 2, P], e4, kind="ExternalInput")
    ones5 = nc.dram_tensor("ones5", [P, 2, P], e5, kind="ExternalInput")
    out = nc.dram_tensor("out", [WPC, S], f32, kind="ExternalOutput")

    from contextlib import ExitStack

    with tile.TileContext(nc) as tc:
        with ExitStack() as ctx:
            ep = ctx.enter_context
            dram = ep(tc.tile_pool(name="dram", bufs=1, space="DRAM"))
            singles = ep(tc.tile_pool(name="singles", bufs=1))
            arena = ep(tc.tile_pool(name="arena", bufs=1))
            hstr = ep(tc.tile_pool(name="hstr", bufs=3))
            xarena = ep(tc.tile_pool(name="xarena", bufs=2))
            wstr = ep(tc.tile_pool(name="wstr", bufs=2))
            pbp = ep(tc.tile_pool(name="pbp", bufs=3))
            p5p = ep(tc.tile_pool(name="p5p", bufs=3))
            atp = ep(tc.tile_pool(name="atp", bufs=2))
            sap = ep(tc.tile_pool(name="sap", bufs=4))
            rbp = ep(tc.tile_pool(name="rbp", bufs=2))
            psum = ep(tc.tile_pool(name="ps", bufs=4, space="PSUM"))
            psB = ep(tc.tile_pool(name="psB", bufs=4, space="PSUM"))

            # ---- DRAM scratch for collectives ----
            arin = [dram.tile([DC, P, SCW], bf, tag=f"arin{j}", name=f"arin{j}")
                    for j in range(SCN)]
            arout = [dram.tile([DC, P, SCW], bf, tag=f"arout{j}", name=f"arout{j}",
                               addr_space="Shared") for j in range(SCN)]
            rsin = [dram.tile([DC, P, SCW], bf, tag=f"rsin{j}", name=f"rsin{j}")
                    for j in range(SCN)]
            rsout = [dram.tile([4, P, SCW], bf, tag=f"rsout{j}", name=f"rsout{j}")
                     for j in range(SCN)]

            ones4_sb = singles.tile([P, 2, P], e4)
            nc.sync.dma_start(out=ones4_sb[:], in_=ones4[:])
            ones5_sb = singles.tile([P, 2, P], e5)
            nc.sync.dma_start(out=ones5_sb[:], in_=ones5[:])
            eps_t = singles.tile([P, 1], f32)
            nc.vector.memset(eps_t[:], EPS)

            # persistent SBUF arenas for q/k/v (fp8)
            qT = arena.tile([P, HPC, S], e4, tag="qT")       # [dh, h, s] = 4*q
            kT = arena.tile([P, HPC, S], e4, tag="kT")       # [dh, h, s] = k
            v8 = arena.tile([P, HPC, 8, 2, DH], e4, tag="v8")  # [kp, h, jj, i, dh] = 8*v

            # ============ phase 1: rmsnorm1 + QKV (per S-chunk) ============
            for j in range(SCN):
                cols = slice(j * SCW, (j + 1) * SCW)
                halves = []
                sq8s = []
                ss = psB.tile([P, SCW], f32, tag="pB", name="ss_ps")
                for hf in range(2):
                    hld = hstr.tile([P, DCH, SCW], bf, tag="hstream",
                                    name=f"hld{hf}")
                    nc.sync.dma_start(
                        out=hld[:],
                        in_=hT[hf * DCH:(hf + 1) * DCH, :, cols].rearrange(
                            "d p s -> p d s"),
                    )
                    halves.append(hld)
                    sq8 = p5p.tile([P, DCH, SCW], e4, tag="p5", name=f"sq8{hf}")
                    nc.vector.tensor_mul(sq8[:], hld[:], hld[:])
                    sq8s.append(sq8)
                    for jj in range(DCH // 2):
                        nc.tensor.matmul(
                            ss[:], lhsT=ones4_sb[:],
                            rhs=sq8[:, 2 * jj:2 * jj + 2, :],
                            start=(hf == 0 and jj == 0),
                            stop=(hf == 1 and jj == DCH // 2 - 1),
                            perf_mode=DR,
                        )
                rbc = rbp.tile([P, SCW], f32, tag="rbc")
                nc.scalar.activation(
                    out=rbc[:], in_=ss[:], func=AF.Sqrt, bias=eps_t[:],
                    scale=1.0 / D,
                )
                nc.vector.reciprocal(out=rbc[:], in_=rbc[:])
                x8 = xarena.tile([P, DC, SCW], e4, tag="x8")
                for d in range(DC):
                    nc.vector.tensor_mul(
                        x8[:, d, :], halves[d // DCH][:, d % DCH, :], rbc[:])

                # ---- Q, K (feature-major out [dh, s]) ----
                for name, wsrc, dst, cscale in (
                    ("q", wq, qT, S_QS / S_WQ), ("k", wk, kT, 1.0 / S_WK),
                ):
                    wsb = wstr.tile([P, DCH, 2, WPC], e4, tag="wstream",
                                    name=f"w{name}sb")
                    nc.sync.dma_start(out=wsb[:], in_=wsrc[:])
                    for h in range(HPC):
                        ps = psum.tile([P, SCW], f32, tag="pA",
                                       name=f"ps_{name}{h}")
                        for dp in range(DCH):
                            nc.tensor.matmul(
                                ps[:], lhsT=wsb[:, dp, :, h * DH:(h + 1) * DH],
                                rhs=x8[:, 2 * dp:2 * dp + 2, :],
                                start=(dp == 0), stop=(dp == DCH - 1),
                                perf_mode=DR,
                            )
                        nc.scalar.mul(dst[:, h, cols], ps[:], cscale)

                # ---- V (natural layout out [s, dh]) ----
                wvsb = wstr.tile([P, DCH, 2, WPC], e4, tag="wstream", name="wvsb")
                nc.sync.dma_start(out=wvsb[:], in_=wv[:])
                for sl in range(SCW // P):
                    ps = psum.tile([P, WPC], f32, tag="pA", name=f"ps_v{sl}")
                    for dp in range(DCH):
                        nc.tensor.matmul(
                            ps[:], lhsT=x8[:, 2 * dp:2 * dp + 2, sl * P:(sl + 1) * P],
                            rhs=wvsb[:, dp, :, :],
                            start=(dp == 0), stop=(dp == DCH - 1),
                            perf_mode=DR,
                        )
                    kcix = j * (SCW // P) + sl
                    nc.scalar.mul(
                        v8[:, :, kcix // 2, kcix % 2, :],
                        ps[:].rearrange("p (h f) -> p h f", h=HPC),
                        S_VS / S_WV,
                    )

            # ============ phase 2: attention (k-major) + WO + AllReduce ====
            for qg in range(SCN):
                qcols = slice(qg * SCW, (qg + 1) * SCW)
                attnT = atp.tile([P, HPC, SCW], e4, tag="attnT")
                for h in range(HPC):
                    p5 = p5p.tile([P, KC, SCW], e5, tag="p5", name="p5")
                    sums = psB.tile([P, SCW], f32, tag="pB", name="sums_ps")
                    for hf in range(2):
                        pbt = pbp.tile([P, 8, SCW], bf, tag="pbt", name="pbt")
                        nc.sync.dma_start(out=pbt[:], in_=pbT[h, qg, hf])
                        for kk in range(8):
                            kc = hf * 8 + kk
                            pss = psum.tile([P, SCW], f32, tag="pA", name="pss")
                            nc.tensor.matmul(
                                pss[:], lhsT=kT[:, h, kc * P:(kc + 1) * P],
                                rhs=qT[:, h, qcols], start=True, stop=True,
                            )
                            sadd = sap.tile([P, SCW], f32, tag="sadd")
                            nc.vector.tensor_add(sadd[:], pss[:], pbt[:, kk, :])
                            nc.scalar.activation(
                                out=p5[:, kc, :], in_=sadd[:], func=AF.Exp,
                                scale=1.0 / S_QS,
                            )
                    for jj in range(KC // 2):
                        nc.tensor.matmul(
                            sums[:], lhsT=ones5_sb[:],
                            rhs=p5[:, 2 * jj:2 * jj + 2, :],
                            start=(jj == 0), stop=(jj == KC // 2 - 1),
                            perf_mode=DR,
                        )
                    psav = psB.tile([P, SCW], f32, tag="pB", name="psav")
                    for jj in range(KC // 2):
                        nc.tensor.matmul(
                            psav[:], lhsT=v8[:, h, jj, :, :],
                            rhs=p5[:, 2 * jj:2 * jj + 2, :],
                            start=(jj == 0), stop=(jj == KC // 2 - 1),
                            perf_mode=DR,
                        )
                    rs = rbp.tile([P, SCW], f32, tag="rbc", name="rs")
                    nc.vector.reciprocal(out=rs[:], in_=sums[:])
                    nc.vector.tensor_mul(attnT[:, h, :], psav[:], rs[:])

                # ---- WO partials for this S chunk -> arin, AllReduce ----
                wosb = wstr.tile([P, 2, 2, D], e4, tag="wstream", name="wosb")
                nc.sync.dma_start(out=wosb[:], in_=wo[:])
                for dg in range(8):
                    stg = p5p.tile([P, 4, SCW], bf, tag="p5", name="wostg")
                    for di in range(4):
                        dcc = dg * 4 + di
                        ps = psum.tile([P, SCW], f32, tag="pA", name="ps_wo")
                        for hp in range(2):
                            nc.tensor.matmul(
                                ps[:],
                                lhsT=wosb[:, hp, :, dcc * P:(dcc + 1) * P],
                                rhs=attnT[:, 2 * hp:2 * hp + 2, :],
                                start=(hp == 0), stop=(hp == 1),
                                perf_mode=DR,
                            )
                        nc.scalar.mul(stg[:, di, :], ps[:], 1.0 / (S_VS * S_WO))
                    nc.sync.dma_start(
                        out=arin[qg][dg * 4:(dg + 1) * 4, :, :].rearrange(
                            "d p s -> p d s"),
                        in_=stg[:],
                    )
                nc.gpsimd.collective_compute(
                    "AllReduce", ALU.add, replica_groups=RG,
                    ins=[arin[qg][:]], outs=[arout[qg][:]],
                )

            # ============ phase 3: h1, rmsnorm2, FFN, ReduceScatter ========
            for sc in range(SCN):
                cols = slice(sc * SCW, (sc + 1) * SCW)
                halves = []
                ss2 = psB.tile([P, SCW], f32, tag="pB", name="ss2_ps")
                for hf in range(2):
                    h1h = hstr.tile([P, DCH, SCW], bf, tag="hstream",
                                    name=f"h1h{hf}")
                    nc.sync.dma_start(
                        out=h1h[:],
                        in_=hT[hf * DCH:(hf + 1) * DCH, :, cols].rearrange(
                            "d p s -> p d s"),
                    )
                    for qr in range(2):
                        ars = pbp.tile([P, 8, SCW], bf, tag="pbt", name="ars")
                        d0 = hf * DCH + qr * 8
                        nc.sync.dma_start(
                            out=ars[:],
                            in_=arout[sc][d0:d0 + 8, :, :].rearrange(
                                "d p s -> p d s"),
                        )
                        nc.vector.tensor_add(
                            h1h[:, qr * 8:(qr + 1) * 8, :],
                            h1h[:, qr * 8:(qr + 1) * 8, :], ars[:])
                    halves.append(h1h)
                    sq8 = p5p.tile([P, DCH, SCW], e4, tag="p5", name=f"fsq8{hf}")
                    nc.vector.tensor_mul(sq8[:], h1h[:], h1h[:])
                    for jj in range(DCH // 2):
                        nc.tensor.matmul(
                            ss2[:], lhsT=ones4_sb[:],
                            rhs=sq8[:, 2 * jj:2 * jj + 2, :],
                            start=(hf == 0 and jj == 0),
                            stop=(hf == 1 and jj == DCH // 2 - 1),
                            perf_mode=DR,
                        )
                rbc2 = rbp.tile([P, SCW], f32, tag="rbc", name="rbc2")
                nc.scalar.activation(
                    out=rbc2[:], in_=ss2[:], func=AF.Sqrt, bias=eps_t[:],
                    scale=1.0 / D,
                )
                nc.vector.reciprocal(out=rbc2[:], in_=rbc2[:])
                for d in range(DC):
                    y = halves[d // DCH][:, d % DCH, :]
                    nc.vector.tensor_mul(y, y, rbc2[:])

                # ---- gated FFN (bf16) ----
                ffT = xarena.tile([P, FCC, SCW], bf, tag="x8", name="ffT")
                for fc in range(FCC):
                    w01b = wstr.tile([P, 2, DC, P], bf, tag="wstream", name="w01b")
                    nc.sync.dma_start(out=w01b[:], in_=w01[fc])
                    psg = psum.tile([P, SCW], f32, tag="pA", name="psg")
                    psu = psum.tile([P, SCW], f32, tag="pA", name="psu")
                    for d in range(DC):
                        y = halves[d // DCH][:, d % DCH, :]
                        nc.tensor.matmul(
                            psg[:], lhsT=w01b[:, 0, d, :], rhs=y,
                            start=(d == 0), stop=(d == DC - 1),
                        )
                        nc.tensor.matmul(
                            psu[:], lhsT=w01b[:, 1, d, :], rhs=y,
                            start=(d == 0), stop=(d == DC - 1),
                        )
                    gel = sap.tile([P, SCW], bf, tag="gel", bufs=2)
                    nc.scalar.activation(out=gel[:], in_=psg[:], func=AF.Gelu)
                    nc.vector.tensor_mul(ffT[:, fc, :], psu[:], gel[:])

                # ---- w_out partials + attention partial -> rsin, RS ----
                for dgp in range(4):
                    arp = pbp.tile([P, 8, SCW], bf, tag="pbt", name="arp")
                    nc.sync.dma_start(
                        out=arp[:],
                        in_=arin[sc][dgp * 8:(dgp + 1) * 8, :, :].rearrange(
                            "d p s -> p d s"),
                    )
                    for dh2 in range(2):
                        dg = dgp * 2 + dh2
                        wob = wstr.tile([P, FCC, 4 * P], bf, tag="wstream",
                                        name="wob")
                        nc.sync.dma_start(out=wob[:], in_=wout[dg])
                        stg = p5p.tile([P, 4, SCW], bf, tag="p5", name="ffstg")
                        for di in range(4):
                            ps = psum.tile([P, SCW], f32, tag="pA", name="ps_o")
                            for fc in range(FCC):
                                nc.tensor.matmul(
                                    ps[:], lhsT=wob[:, fc, di * P:(di + 1) * P],
                                    rhs=ffT[:, fc, :],
                                    start=(fc == 0), stop=(fc == FCC - 1),
                                )
                            nc.vector.tensor_add(
                                stg[:, di, :], ps[:], arp[:, dh2 * 4 + di, :])
                        nc.sync.dma_start(
                            out=rsin[sc][dg * 4:(dg + 1) * 4, :, :].rearrange(
                                "d p s -> p d s"),
                            in_=stg[:],
                        )
                nc.gpsimd.collective_compute(
                    "ReduceScatter", ALU.add, replica_groups=RG,
                    ins=[rsin[sc][:]], outs=[rsout[sc][:]],
                )

            # ============ phase 4: final residual ==========================
            for sc in range(SCN):
                cols = slice(sc * SCW, (sc + 1) * SCW)
                rst = pbp.tile([P, 4, SCW], bf, tag="pbt", name="rst")
                nc.sync.dma_start(
                    out=rst[:], in_=rsout[sc][:].rearrange("o p s -> p o s"))
                hot = hstr.tile([P, 4, SCW], f32, tag="hstream", name="hot")
                nc.sync.dma_start(
                    out=hot[:], in_=h_own[:, :, cols].rearrange("o p s -> p o s"))
                ot = p5p.tile([P, 4, SCW], f32, tag="p5", name="ot")
                nc.vector.tensor_add(ot[:], hot[:], rst[:])
                nc.sync.dma_start(
                    out=out[:, cols].rearrange("(o p) s -> p o s", p=P),
                    in_=ot[:],
                )

    nc.finalize()
    return nc


def _prep_in_maps(inputs):
    import ml_dtypes

    bf16 = ml_dtypes.bfloat16
    e4 = ml_dtypes.float8_e4m3
    e5 = ml_dtypes.float8_e5m2

    def q8(x, s):
        return np.clip(x * s, -240, 240).astype(e4)

    hid = np.ascontiguousarray(np.asarray(inputs["hidden_states"], np.float32)[0])
    mask = np.asarray(inputs["attention_mask"])[0]
    pbias = np.asarray(inputs["position_bias"], np.float32)[0]
    ln_a = np.asarray(inputs["ln_attn_w"], np.float32)
    ln_f = np.asarray(inputs["ln_ffn_w"], np.float32)
    wq = np.asarray(inputs["wq"], np.float32)
    wk = np.asarray(inputs["wk"], np.float32)
    wv = np.asarray(inputs["wv"], np.float32)
    wo = np.asarray(inputs["wo"], np.float32)
    w0 = np.asarray(inputs["w0"], np.float32)
    w1 = np.asarray(inputs["w1"], np.float32)
    w_out = np.asarray(inputs["w_out"], np.float32)

    hT = np.ascontiguousarray(hid.T)                          # (D, S) f32
    hT_bf = hT.reshape(DC, P, S).astype(bf16)
    wq_f = q8(ln_a[:, None] * wq * (DH ** -0.5), S_WQ)
    wk_f = q8(ln_a[:, None] * wk, S_WK)
    wv_f = q8(ln_a[:, None] * wv, S_WV)
    wo_f = q8(wo, S_WO)
    w0_f = (ln_f[:, None] * w0).astype(bf16)
    w1_f = (ln_f[:, None] * w1).astype(bf16)
    wout_f = w_out.astype(bf16)
    if mask.all():
        pb_m = pbias * S_QS
    else:
        pb_m = np.where(mask[None], pbias * S_QS, np.float32(-1e30))
    # transposed position bias: [H, S_k, S_q] -> per-core [HPC,SCN,2,P,8,SCW]
    pbT_full = np.ascontiguousarray(pb_m.transpose(0, 2, 1)).astype(bf16)

    ones4 = np.ones((P, 2, P), dtype=e4)
    ones5 = np.ones((P, 2, P), dtype=e5)

    def wqkv_layout(w):                # (D, WPC) -> (P, DCH, 2, WPC)
        return np.ascontiguousarray(
            w.reshape(DCH, 2, P, WPC).transpose(2, 0, 1, 3))

    in_maps = []
    for c in range(NCORES):
        ws = slice(c * WPC, (c + 1) * WPC)
        fs = slice(c * FPC, (c + 1) * FPC)
        # wo: rows (2hp+i)*128+p of this core's (WPC, D) slice
        wo_c = np.ascontiguousarray(
            wo_f[ws, :].reshape(2, 2, P, D).transpose(2, 0, 1, 3))
        pb_c = pbT_full[c * HPC:(c + 1) * HPC]                # (HPC, S_k, S_q)
        pb_c = pb_c.reshape(HPC, 2, 8, P, SCN, SCW).transpose(0, 4, 1, 3, 2, 5)
        w0_c = w0_f[:, fs].reshape(DC, P, FCC, P).transpose(2, 1, 0, 3)
        w1_c = w1_f[:, fs].reshape(DC, P, FCC, P).transpose(2, 1, 0, 3)
        w01_c = np.stack([w0_c, w1_c], axis=2)
        wout_c = wout_f[fs, :].reshape(FCC, P, 8, 4 * P).transpose(2, 1, 0, 3)
        in_maps.append({
            "hT": hT_bf,
            "h_own": np.ascontiguousarray(hT[ws].reshape(4, P, S)),
            "wq": wqkv_layout(wq_f[:, ws]),
            "wk": wqkv_layout(wk_f[:, ws]),
            "wv": wqkv_layout(wv_f[:, ws]),
            "wo": wo_c,
            "pbT": np.ascontiguousarray(pb_c),
            "w01": np.ascontiguousarray(w01_c),
            "wout": np.ascontiguousarray(wout_c),
            "ones4": ones4,
            "ones5": ones5,
        })
    return in_maps


def get_nc():
    if "nc" not in _CACHE:
        _CACHE["nc"] = _build()
    return _CACHE["nc"]


def kernel(**inputs):
    from concourse.bass_utils import run_bass_kernel_spmd

    nc = get_nc()
    in_maps = _prep_in_maps(inputs)
    res = run_bass_kernel_spmd(nc, in_maps, core_ids=list(range(NCORES)))
    parts = [res.results[c]["out"] for c in range(NCORES)]   # each (WPC, S)
    full_T = np.concatenate(parts, axis=0)                    # (D, S)
    out = np.ascontiguousarray(full_T.T)[None]                # (1, S, D)
    return out.astype(np.float32)
